# revision 51
# baseline (speedup 1.0000x reference)
"""AdaptiveKNN Trainium2 kernel (8 NeuronCores, SPMD).

Sharding: data-parallel over batch B=2 across pairs-of-4 cores; within a
batch, the N=8192 query rows are row-sharded 4 ways (2048 rows/core).
Each core computes its [2048, 8192] distance block, per-row top-128
neighbors (sorted, with indices), the radius count, and (after a tiny
AllReduce for the batch mean density) the adaptive-k mask.

Distance values reproduce the reference f32 rounding exactly:
sb = fl(2g - fl(sq_i + sq_j)) with 2g from a K=3 PE matmul of pre-doubled
query coords (doubling is exact), ssum on the scalar engine (one rounding),
and the subtract on GPSIMD.  Matching the rounding bit-for-bit matters:
the reference's own ~2e-7 cancellation noise otherwise reorders ~2k
near-tied neighbor pairs.

Per 128-query tile, software-pipelined so the in-order DVE queue never
heads-of-line-blocks on the Pool/scalar engines:
  screen(t):  16x [matmul -> ACT psum drain + ssum -> Pool subtract ->
              DVE max8/max_index per 64-wide segment] -> pool of 1024
              exact seg-top-8 values + in-segment positions
  tail_a(t):  q = 1/max(-sb, 1e-9) (positive keys; zero-filled scatter
              slots lose, self becomes exactly 1e9 = row max); per-row tau
              from a 21-level global geometric ladder evaluated as
              sum(sign(q-Lv)) on the scalar engine over the seg-top-3
              subset (largest level with >= 142 survivors; <= ~189 by
              construction, CCAP 208); self dropped via its known pool
              slot 8*(selfidx>>6); rank-compaction via prefix scan and
              two GPSIMD local_scatters (values as u16 pairs)
  tail_b(t-1): 16 rounds of max8+max_index+match_replace emit the sorted
              top-128; permutation inverted with two small local_scatters
              to recover global indices; dist = sqrt(1/q + eps) and the
              radius count per row (partition-all-reduced incrementally).
"""

import os
import sys

sys.path.insert(0, "/opt/trn_rl_repo")

import numpy as np

import concourse.bass as bass
import concourse.bacc as bacc
import concourse.mybir as mybir
from concourse.tile import TileContext
from concourse.bass_utils import run_bass_kernel_spmd

F32 = mybir.dt.float32
U16 = mybir.dt.uint16
I16 = mybir.dt.int16
I32 = mybir.dt.int32
U8 = mybir.dt.uint8
AF = mybir.ActivationFunctionType
OP = mybir.AluOpType

B, N, D = 2, 8192, 64
K_BASE, K_MIN, K_MAX = 32, 8, 128
RADIUS, EPS = 0.05, 1e-8
KOUT = 128

P = 128            # partitions / queries per tile
RPC = 2048         # rows per core
NT = RPC // P      # row tiles per core (16)
NBLK = N // 512    # 512-wide candidate blocks per tile (16)
SEG = 64           # segment size for first-level top-8
NSEG = N // SEG    # 128
POOL = NSEG * 8    # 1024
CCAP = 208         # compaction capacity (ladder count <= ~189)
ROUNDS = 16        # 16*8 = 128 extracted (self already removed)
SEL = ROUNDS * 8   # 128
NEG = -3.0e38
D32 = float(np.float32(N * RADIUS**3 + EPS))  # density denominator, f32
INVD = float(np.float32(1.0 / np.float64(np.float32(D32))))  # XLA-folded 1/D

# Global geometric tau ladder in sb = -d2 space: level k accepts rows
# whose seg-top-3 count of {sb >= -D2LV[k]} is >= 142; the largest
# accepted level leaves 142..~189 survivors (CCAP 208).
# sum(sign(sb+D2LV[k])) >= 2*142-384  <=>  count >= 142.
D2LVS = [float(1.0 / (15.62 * 1.10 ** k)) for k in range(21)]
LVOFF = 4.0        # lvvals = LVOFF - D2LV (positive, ascending in closeness)
SIGN_THR = -100.0


def build_program(sim_single_core=False):
    nc = bacc.Bacc("TRN2", target_bir_lowering=False,
                   num_devices=1 if sim_single_core else 8)

    caug = nc.declare_dram_parameter("caug", [3, N], F32, isOutput=False)
    qaug = nc.declare_dram_parameter("qaug", [3, RPC], F32, isOutput=False)
    sqc = nc.declare_dram_parameter("sqc", [1, N], F32, isOutput=False)
    sqq = nc.declare_dram_parameter("sqq", [P, NT], F32, isOutput=False)
    selfslot = nc.declare_dram_parameter("selfslot", [P, NT], F32, isOutput=False)
    bsel = nc.declare_dram_parameter("bsel", [1, 2], F32, isOutput=False)
    gam = nc.declare_dram_parameter("gamma_param", [1, 1], F32, isOutput=False)
    oidx = nc.declare_dram_parameter("oidx", [RPC, KOUT], I32, isOutput=True)
    odist = nc.declare_dram_parameter("odist", [RPC, KOUT], F32, isOutput=True)
    omask = nc.declare_dram_parameter("omask", [RPC, KOUT], U8, isOutput=True)

    with TileContext(nc) as tc:
        with (
            tc.tile_pool(name="const", bufs=1) as cpool,
            tc.tile_pool(name="sblk", bufs=3) as spool,
            tc.tile_pool(name="poolp", bufs=2) as ppool,
            tc.tile_pool(name="scratch", bufs=2) as scr,
            tc.tile_pool(name="outs", bufs=2) as opool,
            tc.tile_pool(name="masks", bufs=6) as mpool,
            tc.tile_pool(name="psum", bufs=8, space="PSUM") as psum,
            tc.tile_pool(name="dram", bufs=1, space="DRAM") as dpool,
        ):
            # ---- persistent constants / inputs ----
            caug_sb = cpool.tile([3, N], F32)
            qaug_sb = cpool.tile([3, RPC], F32)
            sqq_sb = cpool.tile([P, NT], F32)
            selfslot_sb = cpool.tile([P, NT], F32)
            bsel_sb = cpool.tile([1, 2], F32)
            gam_sb = cpool.tile([1, 1], F32)
            nc.sync.dma_start(out=caug_sb[:], in_=caug[:])
            nc.sync.dma_start(out=qaug_sb[:], in_=qaug[:])

            nc.sync.dma_start(out=sqq_sb[:], in_=sqq[:])
            nc.sync.dma_start(out=selfslot_sb[:], in_=selfslot[:])
            nc.sync.dma_start(out=bsel_sb[:], in_=bsel[:])
            nc.sync.dma_start(out=gam_sb[:], in_=gam[:])

            # constants built on device (iotas emitted after screen(0) so
            # the in-order Pool queue reaches tile 0's subtracts quickly)
            segbase = cpool.tile([P, POOL], U16)       # (pool slot>>3)*64
            iota1u = cpool.tile([P, POOL], U16)        # 1..1024
            epscol = cpool.tile([P, 1], F32)
            nc.vector.memset(epscol[:], EPS)
            iotah32 = cpool.tile([P, KOUT], I32)
            iotah = cpool.tile([P, KOUT], F32)
            sqcb = cpool.tile([P, N], F32)
            nc.sync.dma_start(out=sqcb[0:1, :], in_=sqc[:])
            lvvals = cpool.tile([P, len(D2LVS)], F32)
            nlvvals = cpool.tile([P, len(D2LVS)], F32)
            for k, v in enumerate(D2LVS):
                nc.vector.memset(lvvals[:, k:k + 1], LVOFF - v)
                nc.vector.memset(nlvvals[:, k:k + 1], v)

            cnts = cpool.tile([P, NT], F32)            # radius counts per row
            cnt1 = cpool.tile([P, NT], F32)            # counts + self
            colsum = cpool.tile([P, NT], F32)          # partition-reduced

            # ---- software-pipelined main loop ----
            # Emission order per iteration t:  screen(t) -> tail_a(t)
            # (tau ladder, mask, scan -> Pool compaction chain) ->
            # tail_b(t-1) (top-k rounds, inversion, outputs).  This keeps
            # the in-order DVE queue busy with tile t's screen while the
            # Pool engine compacts tile t-1, avoiding head-of-line stalls.
            state = {}

            def emit_screen(t):
                Pv = ppool.tile([P, POOL], F32, tag="Pv")
                PI = ppool.tile([P, POOL], U16, tag="PI")
                for j in range(NBLK):
                    if t == 0 and j % 4 == 0:
                        cs = slice(j * 512, (j + 4) * 512)
                        nc.gpsimd.partition_broadcast(sqcb[:, cs],
                                                      sqcb[0:1, cs])
                    g_ps = psum.tile([P, 512], F32, tag="g_ps")
                    nc.tensor.matmul(
                        g_ps[:],
                        qaug_sb[:, t * P:(t + 1) * P],
                        caug_sb[:, j * 512:(j + 1) * 512],
                        start=True, stop=True,
                    )
                    # ssum = fl(sq_c + sq_q) on the scalar engine, then
                    # sb = fl(2g - ssum) on GPSIMD -- byte-exact with the
                    # reference rounding; DVE runs the screen from SBUF
                    ssum = spool.tile([P, 512], F32, tag="ssum")
                    nc.scalar.activation(ssum[:], sqcb[:, j * 512:(j + 1) * 512],
                                         AF.Identity, bias=sqq_sb[:, t:t + 1],
                                         scale=1.0)
                    g2 = spool.tile([P, 512], F32, tag="g2")
                    nc.scalar.activation(g2[:], g_ps[:], AF.Identity,
                                         bias=0.0, scale=1.0)
                    kb = spool.tile([P, 512], F32, tag="kb")
                    nc.gpsimd.tensor_sub(kb[:], g2[:], ssum[:])
                    for sg in range(8):
                        seg = 8 * j + sg
                        sl = kb[:, sg * SEG:(sg + 1) * SEG]
                        nc.vector.max(out=Pv[:, seg * 8:(seg + 1) * 8], in_=sl)
                        nc.vector.max_index(PI[:, seg * 8:(seg + 1) * 8],
                                            Pv[:, seg * 8:(seg + 1) * 8], sl)
                state[(t, "Pv")] = Pv
                state[(t, "PI")] = PI

            def emit_tail_a(t):
                Pv = state.pop((t, "Pv"))
                PI = state.pop((t, "PI"))
                OIu = scr.tile([P, POOL], U16, tag="OIu")
                nc.vector.tensor_tensor(OIu[:], PI[:], segbase[:], op=OP.add)

                # tau ladder on the scalar engine: for each global level,
                # sum(sign(sb + D2LV)) over the seg-top-3 subset; the
                # largest level with count >= 142 becomes this row's tau.
                Pq = Pv
                Pq3 = Pq.rearrange("p (s k) -> p s k", k=8)[:, :, 0:3]
                NL = len(D2LVS)
                scols = scr.tile([P, NL], F32, tag="scols")
                sjunk = scr.tile([P, NSEG * 3], F32, tag="sjunk")
                sj3 = sjunk.rearrange("p (s k) -> p s k", k=3)
                for k in range(NL):
                    nc.scalar.activation(sj3[:], Pq3, AF.Sign,
                                         bias=nlvvals[:, k:k + 1], scale=1.0,
                                         accum_out=scols[:, k:k + 1])
                accq = scr.tile([P, NL], F32, tag="accq")
                nc.vector.tensor_scalar(accq[:], scols[:], SIGN_THR, None,
                                        op0=OP.is_ge)
                qv = scr.tile([P, NL], F32, tag="qv")
                nc.vector.tensor_tensor(qv[:], accq[:], lvvals[:], op=OP.mult)
                Lm = scr.tile([P, 1], F32, tag="Lm")
                nc.vector.tensor_reduce(Lm[:], qv[:],
                                        axis=mybir.AxisListType.X, op=OP.max)
                L = scr.tile([P, 1], F32, tag="L")
                nc.vector.tensor_scalar(L[:], Lm[:], -LVOFF, None, op0=OP.add)

                # self sits at pool slot 8*(selfidx>>6) (top-1 of its own
                # segment); smv marks it so the mask drops it pre-compaction.
                smv = scr.tile([P, POOL], F32, tag="smv")
                nc.vector.tensor_scalar(smv[:], iota1u[:],
                                        selfslot_sb[:, t:t + 1], None,
                                        op0=OP.is_equal)
                msk = scr.tile([P, POOL], F32, tag="msk")
                nc.vector.scalar_tensor_tensor(msk[:], Pq[:], L[:, 0:1],
                                               smv[:], op0=OP.is_ge,
                                               op1=OP.subtract)
                rank1 = scr.tile([P, POOL], F32, tag="rank1")
                nc.vector.tensor_tensor_scan(rank1[:], msk[:], msk[:], 0.0,
                                             op0=OP.add, op1=OP.bypass)
                # Pool chain: positions, scatter indices, compaction
                t1 = scr.tile([P, POOL], F32, tag="t1")
                nc.gpsimd.tensor_mul(t1[:], rank1[:], msk[:])
                posc = scr.tile([P, POOL], F32, tag="posc")
                nc.gpsimd.scalar_tensor_tensor(posc[:], rank1[:], float(CCAP),
                                               t1[:], op0=OP.is_le,
                                               op1=OP.mult)
                idxB = scr.tile([P, POOL], I16, tag="idxB")
                nc.vector.tensor_scalar(idxB[:], posc[:], 1.0, None,
                                        op0=OP.subtract)
                idxpair = scr.tile([P, 2 * POOL], I16, tag="idxpair")
                ipv = idxpair.rearrange("p (a b) -> p a b", b=2)
                nc.vector.tensor_scalar(ipv[:, :, 0], posc[:], 2.0, 2.0,
                                        op0=OP.mult, op1=OP.subtract)
                nc.vector.tensor_scalar(ipv[:, :, 1], posc[:], 2.0, 1.0,
                                        op0=OP.mult, op1=OP.subtract)
                OIc = scr.tile([P, CCAP], U16, tag="OIc")
                nc.gpsimd.local_scatter(OIc[:], OIu[:], idxB[:],
                                        channels=P, num_elems=CCAP,
                                        num_idxs=POOL)
                Vc = scr.tile([P, CCAP], F32, tag="Vc")
                nc.gpsimd.local_scatter(Vc.bitcast(U16)[:], Pq.bitcast(U16)[:],
                                        idxpair[:], channels=P,
                                        num_elems=2 * CCAP, num_idxs=2 * POOL)
                state[(t, "OIc")] = OIc
                state[(t, "Vc")] = Vc

            def emit_tail_b(t):
                OIc = state.pop((t, "OIc"))
                Vc = state.pop((t, "Vc"))
                # zero-filled scatter slots -> NEG (real sb is never +-0:
                # self, the only ~0 value, is excluded pre-compaction)
                is0 = scr.tile([P, CCAP], F32, tag="is0")
                nc.vector.tensor_scalar(is0[:], Vc[:], 0.0, None,
                                        op0=OP.is_equal)
                nc.vector.scalar_tensor_tensor(Vc[:], is0[:], NEG, Vc[:],
                                               op0=OP.mult, op1=OP.add)

                # exact top-128 rounds on compacted keys
                vsel = scr.tile([P, SEL], F32, tag="vsel")
                psel = scr.tile([P, SEL], U16, tag="psel")
                for r in range(ROUNDS):
                    nc.vector.max(out=vsel[:, r * 8:(r + 1) * 8], in_=Vc[:])
                    nc.vector.max_index(psel[:, r * 8:(r + 1) * 8],
                                        vsel[:, r * 8:(r + 1) * 8], Vc[:])
                    if r < ROUNDS - 1:
                        nc.vector.match_replace(Vc[:],
                                                vsel[:, r * 8:(r + 1) * 8],
                                                Vc[:], NEG)

                # invert permutation (Pool) while DVE finishes distances
                invp = scr.tile([P, CCAP], U16, tag="invp")
                nc.gpsimd.local_scatter(invp[:], iota1u[:, :SEL],
                                        psel.bitcast(I16)[:],
                                        channels=P, num_elems=CCAP,
                                        num_idxs=SEL)
                d2t = opool.tile([P, KOUT], F32, tag="d2t")
                nc.vector.tensor_scalar(d2t[:], vsel[:], -1.0, 0.0,
                                        op0=OP.mult, op1=OP.max)
                dist = opool.tile([P, KOUT], F32, tag="dist")
                nc.scalar.activation(dist[:], d2t[:], AF.Sqrt,
                                     bias=epscol[:, 0:1], scale=1.0)
                nc.sync.dma_start(out=odist[t * P:(t + 1) * P, :], in_=dist[:])
                junk = opool.tile([P, KOUT], F32, tag="junk")
                nc.vector.tensor_scalar(junk[:], dist[:], RADIUS, 0.0,
                                        op0=OP.is_lt, op1=OP.add,
                                        accum_out=cnts[:, t:t + 1])
                nc.vector.tensor_scalar(cnt1[:, t:t + 1], cnts[:, t:t + 1],
                                        1.0, None, op0=OP.add)
                nc.gpsimd.partition_all_reduce(colsum[:, t:t + 1],
                                               cnt1[:, t:t + 1], channels=P,
                                               reduce_op=bass.bass_isa.ReduceOp.add)
                ipm1 = scr.tile([P, CCAP], I16, tag="ipm1")
                nc.vector.tensor_scalar(ipm1[:], invp[:], 1.0, None,
                                        op0=OP.subtract)
                selo = opool.tile([P, SEL], U16, tag="selo")
                nc.gpsimd.local_scatter(selo[:], OIc[:], ipm1[:],
                                        channels=P, num_elems=SEL,
                                        num_idxs=CCAP)
                oidx32 = opool.tile([P, KOUT], I32, tag="oidx32")
                nc.vector.tensor_copy(oidx32[:], selo[:])
                nc.sync.dma_start(out=oidx[t * P:(t + 1) * P, :], in_=oidx32[:])

            def emit_late_consts():
                nc.gpsimd.iota(segbase[:], pattern=[[SEG, NSEG], [0, 8]],
                               base=0, channel_multiplier=0)
                nc.gpsimd.iota(iota1u[:], pattern=[[1, POOL]], base=1,
                               channel_multiplier=0)
                nc.gpsimd.iota(iotah32[:], pattern=[[1, KOUT]], base=1,
                               channel_multiplier=0)
                nc.vector.tensor_scalar(iotah[:], iotah32[:], 0.5, None,
                                        op0=OP.subtract)

            for t in range(NT):
                emit_screen(t)
                if t == 0:
                    emit_late_consts()
                emit_tail_a(t)
                if t >= 1:
                    emit_tail_b(t - 1)
            emit_tail_b(NT - 1)

            # ---- density, mean (AllReduce), adaptive k, mask ----
            dens = cpool.tile([P, NT], F32)
            nc.vector.tensor_scalar(dens[:], cnt1[:], INVD, None,
                                    op0=OP.mult)
            tot = cpool.tile([P, 1], F32)
            nc.vector.tensor_reduce(tot[:], colsum[:], axis=mybir.AxisListType.X,
                                    op=OP.add)
            payload = cpool.tile([1, 2], F32)
            nc.vector.tensor_scalar(payload[:], bsel_sb[:], tot[0:1, 0:1], None,
                                    op0=OP.mult)
            ccin = dpool.tile([1, 2], F32)
            ccout = dpool.tile([1, 2], F32, addr_space="Shared")
            nc.sync.dma_start(out=ccin[:], in_=payload[:])
            if sim_single_core:
                nc.sync.dma_start(out=ccout[:], in_=ccin[:])
            else:
                nc.gpsimd.collective_compute(
                    "AllReduce", OP.add,
                    replica_groups=[list(range(8))],
                    ins=[ccin.opt()], outs=[ccout.opt()],
                )
            totals = cpool.tile([1, 2], F32)
            nc.sync.dma_start(out=totals[:], in_=ccout[:])
            myt = cpool.tile([1, 2], F32)
            nc.vector.tensor_mul(myt[:], totals[:], bsel_sb[:])
            tot2 = cpool.tile([1, 1], F32)
            nc.vector.tensor_reduce(tot2[:], myt[:], axis=mybir.AxisListType.X,
                                    op=OP.add)
            sdens = cpool.tile([1, 1], F32)
            nc.vector.tensor_scalar(sdens[:], tot2[:], INVD, None,
                                    op0=OP.mult)
            meansc = cpool.tile([1, 1], F32)
            nc.vector.tensor_scalar(meansc[:], sdens[:], 1.0 / N, None,
                                    op0=OP.mult)
            meanb = cpool.tile([P, 1], F32)
            nc.gpsimd.partition_broadcast(meanb[:], meansc[:])
            gsig = cpool.tile([1, 1], F32)
            nc.scalar.activation(gsig[:], gam_sb[:], AF.Sigmoid, bias=0.0,
                                 scale=1.0)
            gb = cpool.tile([P, 1], F32)
            nc.gpsimd.partition_broadcast(gb[:], gsig[:])

            deps = cpool.tile([P, NT], F32)
            nc.vector.tensor_scalar(deps[:], dens[:], EPS, None, op0=OP.add)
            recdep = cpool.tile([P, NT], F32)
            nc.vector.reciprocal(recdep[:], deps[:])
            ratio = cpool.tile([P, NT], F32)
            nc.vector.tensor_scalar(ratio[:], recdep[:], meanb[:, 0:1], None,
                                    op0=OP.mult)
            lnr = cpool.tile([P, NT], F32)
            nc.scalar.activation(lnr[:], ratio[:], AF.Ln, bias=0.0, scale=1.0)
            lng = cpool.tile([P, NT], F32)
            nc.vector.tensor_scalar(lng[:], lnr[:], gb[:, 0:1], None,
                                    op0=OP.mult)
            powr = cpool.tile([P, NT], F32)
            nc.scalar.activation(powr[:], lng[:], AF.Exp, bias=0.0, scale=1.0)
            kv1 = cpool.tile([P, NT], F32)
            nc.vector.tensor_scalar(kv1[:], powr[:], float(K_BASE), float(K_MIN),
                                    op0=OP.mult, op1=OP.max)
            kv = cpool.tile([P, NT], F32)
            nc.vector.tensor_scalar(kv[:], kv1[:], float(K_MAX), None,
                                    op0=OP.min)
            for t in range(NT):
                maskt = mpool.tile([P, KOUT], U8, tag="maskt")
                nc.vector.tensor_scalar(maskt[:], iotah[:],
                                        kv[:, t:t + 1], None, op0=OP.is_le)
                nc.sync.dma_start(out=omask[t * P:(t + 1) * P, :], in_=maskt[:])

    nc.compile()
    return nc


_PROGRAM = None


def _get_program():
    global _PROGRAM
    if _PROGRAM is None:
        _PROGRAM = build_program()
    return _PROGRAM


def make_in_maps(coords, times, features, gamma_param):
    pts = np.concatenate(
        [np.asarray(coords, dtype=np.float32),
         np.asarray(times, dtype=np.float32)[..., None]], axis=-1)  # [B,N,3]
    x, y, tt = pts[..., 0], pts[..., 1], pts[..., 2]
    sq = ((x * x + y * y) + tt * tt).astype(np.float32)             # [B,N]
    gam = np.asarray(gamma_param, dtype=np.float32).reshape(1, 1)

    in_maps = []
    for c in range(8):
        b, r = c // 4, c % 4
        q0 = r * RPC
        # candidate side: [x, y, t]; query side pre-doubled [2x, 2y, 2t]
        # so the K=3 PE matmul yields 2<p_q,p_c> with the same rounding as
        # the reference einsum (doubling is exact in fp32)
        caug = np.ascontiguousarray(pts[b].T)
        qs = slice(q0, q0 + RPC)
        qaug = np.ascontiguousarray(2.0 * pts[b, qs].T).astype(np.float32)
        in_maps.append({
            "caug": caug,
            "qaug": qaug,
            "sqc": np.ascontiguousarray(sq[b][None, :]),
            "sqq": np.ascontiguousarray(sq[b, qs].reshape(NT, P).T),
            "selfslot": np.ascontiguousarray(
                (8.0 * ((q0 + np.arange(RPC)) // SEG) + 1.0)
                .astype(np.float32).reshape(NT, P).T),
            "bsel": np.eye(2, dtype=np.float32)[b][None, :],
            "gamma_param": gam.copy(),
        })
    return in_maps


def assemble(results):
    idx = np.empty((B, N, KOUT), np.int32)
    msk = np.empty((B, N, KOUT), bool)
    dst = np.empty((B, N, KOUT), np.float32)
    for c in range(8):
        b, r = c // 4, c % 4
        q0 = r * RPC
        idx[b, q0:q0 + RPC] = results[c]["oidx"]
        msk[b, q0:q0 + RPC] = results[c]["omask"].astype(bool)
        dst[b, q0:q0 + RPC] = results[c]["odist"]
    return idx, msk, dst


_EXEC = None


def _get_executor():
    """Build the SPMD program and a jit-once PJRT callable (cached)."""
    global _EXEC
    if _EXEC is not None:
        return _EXEC
    import jax
    from concourse.bass2jax import (_bass_exec_p, install_neuronx_cc_hook,
                                    partition_id_tensor)
    from jax.sharding import Mesh, PartitionSpec
    from jax.experimental.shard_map import shard_map

    nc = _get_program()
    install_neuronx_cc_hook()
    partition_name = nc.partition_id_tensor.name if nc.partition_id_tensor else None
    in_names, out_names, out_avals, zero_shapes = [], [], [], []
    for alloc in nc.m.functions[0].allocations:
        if not isinstance(alloc, mybir.MemoryLocationSet):
            continue
        name = alloc.memorylocations[0].name
        if alloc.kind == "ExternalInput":
            if name != partition_name:
                in_names.append(name)
        elif alloc.kind == "ExternalOutput":
            shape = tuple(alloc.tensor_shape)
            dt = mybir.dt.np(alloc.dtype)
            out_avals.append(jax.core.ShapedArray(shape, dt))
            out_names.append(name)
            zero_shapes.append((shape, dt))
    n_params = len(in_names)
    n_outs = len(out_avals)
    param_names = list(in_names)
    in_names = in_names + out_names
    if partition_name is not None:
        in_names.append(partition_name)

    def _body(*args):
        operands = list(args)
        if partition_name is not None:
            operands.append(partition_id_tensor())
        outs = _bass_exec_p.bind(
            *operands, out_avals=tuple(out_avals), in_names=tuple(in_names),
            out_names=tuple(out_names),
            lowering_input_output_aliases=tuple(
                {i: n_params + i for i in range(n_outs)}.items()),
            sim_require_finite=True, sim_require_nnan=True, nc=nc)
        return tuple(outs)

    n_cores = 8
    devices = jax.devices()[:n_cores]
    mesh = Mesh(np.asarray(devices), ("core",))
    fn = jax.jit(shard_map(_body, mesh=mesh,
                           in_specs=(PartitionSpec("core"),) * (n_params + n_outs),
                           out_specs=(PartitionSpec("core"),) * n_outs,
                           check_rep=False),
                 keep_unused=True)

    def run(in_maps):
        per_core = [[np.ascontiguousarray(m[name]) for name in param_names]
                    for m in in_maps]
        concat_in = [np.concatenate([per_core[c][i] for c in range(n_cores)],
                                    axis=0) for i in range(n_params)]
        concat_zeros = [np.zeros((n_cores * s[0], *s[1:]), d)
                        for s, d in zero_shapes]
        outs = fn(*concat_in, *concat_zeros)
        outs_np = [np.asarray(o) for o in outs]
        results = []
        for c in range(n_cores):
            d = {}
            for i, name in enumerate(out_names):
                per = outs_np[i].shape[0] // n_cores
                d[name] = outs_np[i][c * per:(c + 1) * per]
            results.append(d)
        return results

    _EXEC = run
    return _EXEC


def kernel(coords, times, features, gamma_param):
    run = _get_executor()
    in_maps = make_in_maps(coords, times, features, gamma_param)
    return assemble(run(in_maps))
